# revision 28
# baseline (speedup 1.0000x reference)
"""Causal self-attention (QKV projection + softmax(QK^T/sqrt(N)) @ V) on 8 TRN2
NeuronCores.

Sharding: core c = 2*b + j handles batch element b (of 4) and half its query
rows as four 256-row blocks, interleaved for causal load balance:
  j=0 -> blocks [7,4,3,0], j=1 -> blocks [6,5,2,1]   (block i = rows 256i..256i+255)
Block i needs k-tiles 0..2i+1 (2i+2 of them).  The SPMD schedule runs four
query slots with fixed k-tile counts KS=[16,12,8,4] (the per-rank max over
both cores' sorted needs); per-core causal masks (from shipped position
vectors) zero invalid/extra tiles.  Only the last 4 k-tiles of each slot can
ever be partial/invalid, so exactly those are masked.

Everything is bf16 (rel-err budget is 2e-2; bf16 lands ~2e-3): halves DMA
traffic and SBUF footprint so both K^T and V stay SBUF-resident for the whole
kernel -- no DRAM staging roundtrip -- and enables FWL fast weight loads.

Layout trick (from the fp32r ancestor): context ships pre-transposed [D, N] so
Q^T / K^T come out of the projection directly as [e, n] tiles and V as
[n, e]; scores are computed transposed S^T[k, q] = K^T.T @ Q^T, softmax runs
without max-subtraction (logits are O(1)), the per-query denominator comes
from a ones-vector matmul, and the exp'd S^T is exactly the lhsT that PV
needs.  Zero on-chip transposes.
"""

import math
from contextlib import ExitStack

import numpy as np

import concourse.bass as bass
import concourse.mybir as mybir
import concourse.tile as tile
from concourse.bass_utils import run_bass_kernel_spmd
from concourse.tile_rust import add_dep_helper

P = 128
CH = 512          # free-dim chunk for projection matmuls (one PSUM bank, fp32)
QS = 256          # query rows per attention slot
KS = [16, 12, 8, 4]          # k-tiles per slot (uniform SPMD schedule)
MASK_TILES = 4               # last 4 k-tiles of every slot get masked
BLOCKS = ([7, 4, 3, 0], [6, 5, 2, 1])   # 256-row block ids per core parity


def _chunks(total, size):
    return [(o, min(size, total - o)) for o in range(0, total, size)]


def _fix_matmul_waits(nc):
    """Walrus codegen has a small per-instruction sync-wait slot budget (one
    for a self-loading matmul's LDWEIGHTS half, similar for ACT etc).  Move
    extra waits onto NoOps inserted just before the instruction on the same
    engine -- per-engine program order (and thus semantics) is unchanged."""
    skip = (mybir.InstEventSemaphore, mybir.InstNoOp,
            mybir.InstUnconditionalBranch, mybir.InstCall)
    for func in nc.m.functions:
        for bb in func.blocks:
            il = bb.instructions
            new = []
            changed = False
            for inst in il:
                si = getattr(inst, "sync_info", None)
                if (si and si.on_wait and len(si.on_wait) > 1
                        and not isinstance(inst, skip)):
                    waits = list(si.on_wait)
                    for wi, w in enumerate(waits[:-1]):
                        nop = mybir.InstNoOp(
                            name=f"{inst.name}-wfix{wi}", engine=inst.engine,
                            sync_info=mybir.SyncInfo(on_wait=[w], on_update=[]),
                            text_hint="waitfix")
                        new.append(nop)
                    inst.sync_info = mybir.SyncInfo(
                        on_wait=[waits[-1]], on_update=list(si.on_update or []))
                    changed = True
                new.append(inst)
            if changed:
                bb.instructions = new


def build(N=2048, D=1024, fix_waits=True, **bass_kwargs):
    NT = N // P          # number of 128-row key tiles (16)
    DN = D // P          # contraction tiles / e-tiles (8)
    QTOT = N // 2        # query rows per core (1024)
    NSLOT = QTOT // QS   # attention slots (4)
    SCALE = 1.0 / math.sqrt(N)
    BF = mybir.dt.bfloat16
    F32 = mybir.dt.float32
    AF = mybir.ActivationFunctionType
    OP = mybir.AluOpType

    nc = bass.Bass(**bass_kwargs)
    anchors = []  # first K-proj matmul of each ctx chunk; DMA stage gates
    kends = []    # last K-proj matmul of each ctx chunk

    def _after(dma_bi, anchor_idx, lst=None):
        """Gate a bulk DMA behind an earlier compute anchor so concurrent
        transfers don't fair-share-starve the startup-critical ones."""
        lst = anchors if lst is None else lst
        if lst and anchor_idx < len(lst):
            add_dep_helper(dma_bi.ins, lst[anchor_idx].ins, sync=True,
                           reason="dma staging")
        return dma_bi

    ctx_kvT = nc.declare_dram_parameter("ctx_kvT", [D, N], BF, isOutput=False)
    ctx_qT = nc.declare_dram_parameter("ctx_qT", [D, QTOT], BF, isOutput=False)
    w_qkv = nc.declare_dram_parameter("w_qkv", [D, 3 * D], BF, isOutput=False)
    qpos = nc.declare_dram_parameter("qpos", [P, QTOT], F32, isOutput=False)
    kpos = nc.declare_dram_parameter("kpos", [P, NT], F32, isOutput=False)
    bqT = nc.declare_dram_parameter("bqT", [P, DN], F32, isOutput=False)
    bkT = nc.declare_dram_parameter("bkT", [P, DN], F32, isOutput=False)
    bvb = nc.declare_dram_parameter("bvb", [P, D], F32, isOutput=False)
    onesd = nc.declare_dram_parameter("onesd", [P, 8], BF, isOutput=False)
    out_ext = nc.declare_dram_parameter("out", [QTOT, D], BF, isOutput=True)

    with ExitStack() as ctx:
        tc = ctx.enter_context(tile.TileContext(nc))
        const = ctx.enter_context(tc.tile_pool(name="const", bufs=1))
        persist = ctx.enter_context(tc.tile_pool(name="persist", bufs=1))

        # PE warmup: dummy matmuls on a memset tile bridge the DMA-dead
        # startup window so the HAM clock gate is at 8/8 when real data lands
        # (otherwise the first ~10us of projection run at 1.2 GHz).  Rotating
        # PSUM tiles keep the stream gap-free so the busy-window flips early.
        with tc.tile_pool(name="warm", bufs=1) as warmp, \
             tc.tile_pool(name="warmps", bufs=4, space="PSUM") as warmpp:
            wtile = warmp.tile([P, 2 * P], BF)
            nc.vector.memset(wtile, 0.0)
            for _ in range(64):
                wps = warmpp.tile([P, 2 * P], F32, tag="w", name="wps")
                nc.tensor.matmul(wps, lhsT=wtile[:, 0:P], rhs=wtile, start=True, stop=True)

        bq_sb = const.tile([P, DN], F32)
        nc.sync.dma_start(out=bq_sb, in_=bqT[:, :])
        bk_sb = const.tile([P, DN], F32)
        nc.sync.dma_start(out=bk_sb, in_=bkT[:, :])
        ones_sb = const.tile([P, 8], BF)
        nc.sync.dma_start(out=ones_sb, in_=onesd[:, :])
        kpos_sb = const.tile([P, NT], F32)
        nc.sync.dma_start(out=kpos_sb, in_=kpos[:, :])
        qpos_sb = const.tile([P, QTOT], F32)
        bv_sb = const.tile([P, D], F32)

        # K^T and V both SBUF-resident for the whole kernel (bf16 makes room).
        keT = [persist.tile([P, N], BF, tag=f"ke{e}", name=f"ke{e}") for e in range(DN)]
        v_sb = [persist.tile([P, D], BF, tag=f"v{t}", name=f"v{t}") for t in range(NT)]

        # Q-phase tiles live in outer pools so their DMAs can be issued early,
        # interleaved with the KV-phase transfers on the queue.
        wqp = ctx.enter_context(tc.tile_pool(name="wq", bufs=1))
        ctxq = ctx.enter_context(tc.tile_pool(name="ctxq", bufs=1))
        wq_sb = [wqp.tile([P, D], BF, tag=f"wq{d}", name=f"wq{d}") for d in range(DN)]
        cq_sb = [ctxq.tile([P, QTOT], BF, tag=f"cq{d}", name=f"cq{d}") for d in range(DN)]

        # ---------------- K/V projection (ctx_kvT read once) ----------------
        # Two passes: all K-proj chunks first, then all V-proj.  This defers
        # the W_v deadline by ~40us, giving the startup DMA crunch (wk + early
        # ctx chunks at ~170GB/s under 8-core HBM contention) room to breathe.
        with tc.tile_pool(name="wkv", bufs=1) as wkv, \
             tc.tile_pool(name="ctxp", bufs=1) as ctxp, \
             tc.tile_pool(name="pp", bufs=8, space="PSUM") as pp:
            wk_sb = [wkv.tile([P, D], BF, tag=f"wk{d}", name=f"wk{d}") for d in range(DN)]
            wv_sb = [wkv.tile([P, D], BF, tag=f"wv{d}", name=f"wv{d}") for d in range(DN)]
            # startup-critical DMA order: W_k first halves (sync ring), first
            # ctx chunk (gpsimd ring, in parallel), then the rest chained
            HF = D // 2
            for d in range(DN):
                nc.sync.dma_start(out=wk_sb[d][:, 0:HF], in_=w_qkv[d * P:(d + 1) * P, D:D + HF])
            first_cts = []
            for d in range(DN):
                ct = ctxp.tile([P, CH], BF, tag=f"ct0_{d}", name=f"ct0_{d}")
                stage0_last = nc.gpsimd.dma_start(ct, ctx_kvT[d * P:(d + 1) * P, 0:CH])
                first_cts.append(ct)
            for d in range(DN):
                wk2 = nc.sync.dma_start(out=wk_sb[d][:, HF:D], in_=w_qkv[d * P:(d + 1) * P, D + HF:2 * D])
                add_dep_helper(wk2.ins, stage0_last.ins, sync=True, reason="dma staging")
            for d in range(DN):
                wvd = nc.sync.dma_start(out=wv_sb[d], in_=w_qkv[d * P:(d + 1) * P, 2 * D:3 * D])
                add_dep_helper(wvd.ins, wk2.ins, sync=True, reason="dma staging")
            bvd = nc.sync.dma_start(out=bv_sb, in_=bvb[:, :])
            add_dep_helper(bvd.ins, wvd.ins, sync=True, reason="dma staging")

            chunks = _chunks(N, CH)
            for ci, (coff, csz) in enumerate(chunks):
                if ci == 0:
                    cts = first_cts
                else:
                    cts = []
                    for d in range(DN):
                        ct = ctxp.tile([P, CH], BF, tag=f"ct{ci}_{d}", name=f"ct{ci}_{d}")
                        _after(nc.sync.dma_start(out=ct[:, :csz],
                                                 in_=ctx_kvT[d * P:(d + 1) * P, coff:coff + csz]), ci - 1)
                        cts.append(ct)
                    if ci == 1:        # Q-phase weights: land by ~mid-KV
                        for d in range(DN):
                            _after(nc.sync.dma_start(out=wq_sb[d], in_=w_qkv[d * P:(d + 1) * P, 0:D]), 0)
                    if ci == 2:        # Q-phase context + positions
                        for d in range(DN):
                            _after(nc.sync.dma_start(out=cq_sb[d], in_=ctx_qT[d * P:(d + 1) * P, :]), 1)
                        _after(nc.sync.dma_start(out=qpos_sb, in_=qpos[:, :]), 1)
                for e in range(DN):
                    psk = pp.tile([P, CH], F32, tag="pp8", name="psk")
                    for d in range(DN):
                        mm = nc.tensor.matmul(psk[:, :csz], lhsT=wk_sb[d][:, e * P:(e + 1) * P],
                                              rhs=cts[d][:, :csz], start=(d == 0), stop=(d == DN - 1))
                        if e == 0 and d == 0:
                            anchors.append(mm)
                        if e == DN - 1 and d == DN - 1:
                            kends.append(mm)
                    nc.scalar.activation(keT[e][:, coff:coff + csz], psk[:, :csz],
                                         AF.Identity, bias=bk_sb[:, e:e + 1], scale=1.0)
                for nt_loc in range(csz // P):
                    n_t = coff // P + nt_loc
                    for eoff, esz in _chunks(D, CH):
                        psv = pp.tile([P, CH], F32, tag="pp8", name="psv")
                        for d in range(DN):
                            nc.tensor.matmul(psv[:, :esz],
                                             lhsT=cts[d][:, nt_loc * P:(nt_loc + 1) * P],
                                             rhs=wv_sb[d][:, eoff:eoff + esz], start=(d == 0), stop=(d == DN - 1))
                        nc.vector.tensor_tensor(v_sb[n_t][:, eoff:eoff + esz], psv[:, :esz],
                                                bv_sb[:, eoff:eoff + esz], OP.add)

        # ---------------- Q projection + attention slots ----------------
        with tc.tile_pool(name="qtb", bufs=1) as qtb, \
             tc.tile_pool(name="att_e", bufs=2) as epool, \
             tc.tile_pool(name="att_m", bufs=3) as mpool, \
             tc.tile_pool(name="att_o", bufs=3) as opool, \
             tc.tile_pool(name="ps_s", bufs=2, space="PSUM") as ps_s, \
             tc.tile_pool(name="ps_pv", bufs=4, space="PSUM") as ps_pv, \
             tc.tile_pool(name="ps_den", bufs=2, space="PSUM") as ps_den:
            # Q^T for all 1024 local query rows, in [e, q] layout
            qT_sb = [qtb.tile([P, QTOT], BF, tag=f"qtb{e}", name=f"qtb{e}") for e in range(DN)]
            for qoff, qsz in _chunks(QTOT, CH):
                for e in range(DN):
                    psq = ps_s.tile([P, CH], F32, tag="s", name="psq")
                    for d in range(DN):
                        nc.tensor.matmul(psq[:, :qsz], lhsT=wq_sb[d][:, e * P:(e + 1) * P],
                                         rhs=cq_sb[d][:, qoff:qoff + qsz], start=(d == 0), stop=(d == DN - 1))
                    nc.scalar.activation(qT_sb[e][:, qoff:qoff + qsz], psq[:, :qsz],
                                         AF.Identity, bias=bq_sb[:, e:e + 1], scale=1.0)

            for s in range(NSLOT):
                KT = KS[s]
                qr0 = s * QS
                e_sb = [epool.tile([P, QS], BF, tag=f"e{k}", name=f"e{k}") for k in range(KT)]
                # scores + exp (+ mask on the last MASK_TILES k-tiles)
                for k in range(KT):
                    pss = ps_s.tile([P, QS], F32, tag="s", name="pss")
                    for d in range(DN):
                        nc.tensor.matmul(pss, lhsT=keT[d][:, k * P:(k + 1) * P],
                                         rhs=qT_sb[d][:, qr0:qr0 + QS], start=(d == 0), stop=(d == DN - 1))
                    nc.scalar.activation(e_sb[k], pss, AF.Exp, scale=SCALE)
                    if k >= KT - MASK_TILES:
                        m = mpool.tile([P, QS], BF, tag="m", name="m")
                        nc.vector.tensor_scalar(m, qpos_sb[:, qr0:qr0 + QS],
                                                kpos_sb[:, k:k + 1], None, OP.is_ge)
                        nc.vector.tensor_tensor(e_sb[k], e_sb[k], m, OP.mult)
                # PV per 128-row q-tile (V is SBUF-resident: no DMA here).
                # Denominator first, then e-chunk 0 (scaled on ScalarE and
                # DMA'd while e-chunk 1 is still in the matmul pipe), then
                # e-chunk 1 (VectorE) -- keeps the end-of-kernel chain short.
                for qt in range(QS // P):
                    psd = ps_den.tile([P, 8], F32, tag="den", name="psd")
                    for k in range(KT):
                        nc.tensor.matmul(psd, lhsT=e_sb[k][:, qt * P:(qt + 1) * P], rhs=ones_sb,
                                         start=(k == 0), stop=(k == KT - 1))
                    rec = mpool.tile([P, 1], F32, tag="rec", name="rec")
                    nc.vector.reciprocal(rec, psd[:, 0:1])
                    final = (s == NSLOT - 1) and (qt == QS // P - 1)
                    for ei, (eoff, esz) in enumerate(_chunks(D, CH)):
                        pso = ps_pv.tile([P, CH], F32, tag="pv", name="pso")
                        for k in range(KT):
                            nc.tensor.matmul(pso[:, :esz], lhsT=e_sb[k][:, qt * P:(qt + 1) * P],
                                             rhs=v_sb[k][:, eoff:eoff + esz],
                                             start=(k == 0), stop=(k == KT - 1))
                        ot = opool.tile([P, CH], BF, tag="o", name="ot")
                        orow = out_ext[qr0 + qt * P:qr0 + (qt + 1) * P, :]
                        if final and ei == 1:
                            # very last chunk: split scale across both engines
                            # and the store across four DMA rings to shorten
                            # the end-of-kernel serial chain
                            h = esz // 2
                            nc.scalar.activation(ot[:, :h], pso[:, :h], AF.Identity, scale=rec)
                            nc.vector.tensor_scalar_mul(ot[:, h:esz], pso[:, h:esz], rec)
                            q3 = (esz // 128) // 3 * 128
                            rings = ((nc.sync, 0, q3), (nc.gpsimd, q3, 2 * q3),
                                     (nc.scalar, 2 * q3, esz))
                            for ring, lo, hi in rings:
                                ring.dma_start(out=orow[:, eoff + lo:eoff + hi],
                                               in_=ot[:, lo:hi])
                        elif ei == 0:
                            nc.scalar.activation(ot[:, :esz], pso[:, :esz], AF.Identity, scale=rec)
                            nc.gpsimd.dma_start(out=orow[:, eoff:eoff + esz], in_=ot[:, :esz])
                        else:
                            nc.vector.tensor_scalar_mul(ot[:, :esz], pso[:, :esz], rec)
                            nc.sync.dma_start(out=orow[:, eoff:eoff + esz], in_=ot[:, :esz])
    if fix_waits:
        _fix_matmul_waits(nc)
    return nc


def make_in_maps(context, W_qkv, b_qkv, n_cores=8):
    import ml_dtypes
    bf16 = ml_dtypes.bfloat16
    context = np.ascontiguousarray(np.asarray(context, np.float32))
    W_qkv = np.ascontiguousarray(np.asarray(W_qkv, np.float32))
    b_qkv = np.ascontiguousarray(np.asarray(b_qkv, np.float32))
    B, N, D = context.shape
    NT = N // P
    DN = D // P
    kpos = (np.arange(NT)[None, :] * P + np.arange(P)[:, None]).astype(np.float32)
    kpos = np.ascontiguousarray(kpos)
    bq = np.ascontiguousarray(b_qkv[0:D].reshape(DN, P).T)
    bk = np.ascontiguousarray(b_qkv[D:2 * D].reshape(DN, P).T)
    bv = np.ascontiguousarray(np.broadcast_to(b_qkv[2 * D:3 * D], (P, D)))
    w_bf = np.ascontiguousarray(W_qkv.astype(bf16))
    in_maps = []
    for c in range(n_cores):
        b, j = divmod(c, 2)
        blocks = BLOCKS[j]
        ctx_b = context[b]
        ctx_kvT = np.ascontiguousarray(ctx_b.T.astype(bf16))
        rows = np.concatenate([np.arange(i * QS, (i + 1) * QS) for i in blocks])
        ctx_qT = np.ascontiguousarray(ctx_b[rows].T.astype(bf16))
        qpos_b = np.ascontiguousarray(
            np.broadcast_to(rows.astype(np.float32), (P, rows.size)))
        in_maps.append({
            "ctx_kvT": ctx_kvT, "ctx_qT": ctx_qT, "w_qkv": w_bf,
            "qpos": qpos_b, "kpos": kpos, "bqT": bq, "bkT": bk, "bvb": bv,
            "onesd": np.ones((P, 8), bf16),
        })
    return in_maps


def assemble(results, B, N, D):
    out = np.zeros((B, N, D), np.float32)
    for c, res in enumerate(results):
        b, j = divmod(c, 2)
        o = np.asarray(res["out"], np.float32)
        for s, i in enumerate(BLOCKS[j]):
            out[b, i * QS:(i + 1) * QS] = o[s * QS:(s + 1) * QS]
    return out


def run(inputs, trace=False, **spmd_kwargs):
    context = np.asarray(inputs["context"])
    B, N, D = context.shape
    nc = build(N, D)
    in_maps = make_in_maps(context, inputs["W_qkv"], inputs["b_qkv"], n_cores=8)
    res = run_bass_kernel_spmd(nc, in_maps, core_ids=list(range(8)), trace=trace, **spmd_kwargs)
    out = assemble(res.results, B, N, D)
    return out, res


def kernel(context, W_qkv, b_qkv):
    out, _ = run({"context": context, "W_qkv": W_qkv, "b_qkv": b_qkv})
    return out


# revision 29
# speedup vs baseline: 1.0182x; 1.0182x over previous
"""Causal self-attention (QKV projection + softmax(QK^T/sqrt(N)) @ V) on 8 TRN2
NeuronCores.

Sharding: core c = 2*b + j handles batch element b (of 4) and half its query
rows as four 256-row blocks, interleaved for causal load balance:
  j=0 -> blocks [7,4,3,0], j=1 -> blocks [6,5,2,1]   (block i = rows 256i..256i+255)
Block i needs k-tiles 0..2i+1 (2i+2 of them).  The SPMD schedule runs four
query slots with fixed k-tile counts KS=[16,12,8,4] (the per-rank max over
both cores' sorted needs); per-core causal masks (from shipped position
vectors) zero invalid/extra tiles.  Only the last 4 k-tiles of each slot can
ever be partial/invalid, so exactly those are masked.

Everything is bf16 (rel-err budget is 2e-2; bf16 lands ~2e-3): halves DMA
traffic and SBUF footprint so both K^T and V stay SBUF-resident for the whole
kernel -- no DRAM staging roundtrip -- and enables FWL fast weight loads.

Layout trick (from the fp32r ancestor): context ships pre-transposed [D, N] so
Q^T / K^T come out of the projection directly as [e, n] tiles and V as
[n, e]; scores are computed transposed S^T[k, q] = K^T.T @ Q^T, softmax runs
without max-subtraction (logits are O(1)), the per-query denominator comes
from a ones-vector matmul, and the exp'd S^T is exactly the lhsT that PV
needs.  Zero on-chip transposes.
"""

import math
from contextlib import ExitStack

import numpy as np

import concourse.bass as bass
import concourse.mybir as mybir
import concourse.tile as tile
from concourse.bass_utils import run_bass_kernel_spmd
from concourse.tile_rust import add_dep_helper

P = 128
CH = 512          # free-dim chunk for projection matmuls (one PSUM bank, fp32)
QS = 256          # query rows per attention slot
KS = [16, 12, 8, 4]          # k-tiles per slot (uniform SPMD schedule)
MASK_TILES = 4               # last 4 k-tiles of every slot get masked
BLOCKS = ([7, 4, 3, 0], [6, 5, 2, 1])   # 256-row block ids per core parity


def _chunks(total, size):
    return [(o, min(size, total - o)) for o in range(0, total, size)]


def _fix_matmul_waits(nc):
    """Walrus codegen has a small per-instruction sync-wait slot budget (one
    for a self-loading matmul's LDWEIGHTS half, similar for ACT etc).  Move
    extra waits onto NoOps inserted just before the instruction on the same
    engine -- per-engine program order (and thus semantics) is unchanged."""
    skip = (mybir.InstEventSemaphore, mybir.InstNoOp,
            mybir.InstUnconditionalBranch, mybir.InstCall)
    for func in nc.m.functions:
        for bb in func.blocks:
            il = bb.instructions
            new = []
            changed = False
            for inst in il:
                si = getattr(inst, "sync_info", None)
                if (si and si.on_wait and len(si.on_wait) > 1
                        and not isinstance(inst, skip)):
                    waits = list(si.on_wait)
                    for wi, w in enumerate(waits[:-1]):
                        nop = mybir.InstNoOp(
                            name=f"{inst.name}-wfix{wi}", engine=inst.engine,
                            sync_info=mybir.SyncInfo(on_wait=[w], on_update=[]),
                            text_hint="waitfix")
                        new.append(nop)
                    inst.sync_info = mybir.SyncInfo(
                        on_wait=[waits[-1]], on_update=list(si.on_update or []))
                    changed = True
                new.append(inst)
            if changed:
                bb.instructions = new


def build(N=2048, D=1024, fix_waits=True, **bass_kwargs):
    NT = N // P          # number of 128-row key tiles (16)
    DN = D // P          # contraction tiles / e-tiles (8)
    QTOT = N // 2        # query rows per core (1024)
    NSLOT = QTOT // QS   # attention slots (4)
    SCALE = 1.0 / math.sqrt(N)
    BF = mybir.dt.bfloat16
    F32 = mybir.dt.float32
    AF = mybir.ActivationFunctionType
    OP = mybir.AluOpType

    nc = bass.Bass(**bass_kwargs)
    anchors = []  # first K-proj matmul of each ctx chunk; DMA stage gates
    kends = []    # last K-proj matmul of each ctx chunk

    def _after(dma_bi, anchor_idx, lst=None):
        """Gate a bulk DMA behind an earlier compute anchor so concurrent
        transfers don't fair-share-starve the startup-critical ones."""
        lst = anchors if lst is None else lst
        if lst and anchor_idx < len(lst):
            add_dep_helper(dma_bi.ins, lst[anchor_idx].ins, sync=True,
                           reason="dma staging")
        return dma_bi

    ctx_kvT = nc.declare_dram_parameter("ctx_kvT", [D, N], BF, isOutput=False)
    ctx_qT = nc.declare_dram_parameter("ctx_qT", [D, QTOT], BF, isOutput=False)
    w_qkv = nc.declare_dram_parameter("w_qkv", [D, 3 * D], BF, isOutput=False)
    qpos = nc.declare_dram_parameter("qpos", [P, QTOT], F32, isOutput=False)
    kpos = nc.declare_dram_parameter("kpos", [P, NT], F32, isOutput=False)
    bqT = nc.declare_dram_parameter("bqT", [P, DN], F32, isOutput=False)
    bkT = nc.declare_dram_parameter("bkT", [P, DN], F32, isOutput=False)
    bvb = nc.declare_dram_parameter("bvb", [P, D], F32, isOutput=False)
    onesd = nc.declare_dram_parameter("onesd", [P, 8], BF, isOutput=False)
    out_ext = nc.declare_dram_parameter("out", [QTOT, D], BF, isOutput=True)

    with ExitStack() as ctx:
        tc = ctx.enter_context(tile.TileContext(nc))
        const = ctx.enter_context(tc.tile_pool(name="const", bufs=1))
        persist = ctx.enter_context(tc.tile_pool(name="persist", bufs=1))

        # PE warmup: dummy matmuls on a memset tile bridge the DMA-dead
        # startup window so the HAM clock gate is at 8/8 when real data lands
        # (otherwise the first ~10us of projection run at 1.2 GHz).  Rotating
        # PSUM tiles keep the stream gap-free so the busy-window flips early.
        with tc.tile_pool(name="warm", bufs=1) as warmp, \
             tc.tile_pool(name="warmps", bufs=4, space="PSUM") as warmpp:
            wtile = warmp.tile([P, 2 * P], BF)
            nc.vector.memset(wtile, 0.0)
            for _ in range(64):
                wps = warmpp.tile([P, 2 * P], F32, tag="w", name="wps")
                nc.tensor.matmul(wps, lhsT=wtile[:, 0:P], rhs=wtile, start=True, stop=True)

        bq_sb = const.tile([P, DN], F32)
        nc.sync.dma_start(out=bq_sb, in_=bqT[:, :])
        bk_sb = const.tile([P, DN], F32)
        nc.sync.dma_start(out=bk_sb, in_=bkT[:, :])
        ones_sb = const.tile([P, 8], BF)
        nc.sync.dma_start(out=ones_sb, in_=onesd[:, :])
        kpos_sb = const.tile([P, NT], F32)
        nc.sync.dma_start(out=kpos_sb, in_=kpos[:, :])
        qpos_sb = const.tile([P, QTOT], F32)
        bv_sb = const.tile([P, D], F32)

        # K^T and V both SBUF-resident for the whole kernel (bf16 makes room).
        keT = [persist.tile([P, N], BF, tag=f"ke{e}", name=f"ke{e}") for e in range(DN)]
        v_sb = [persist.tile([P, D], BF, tag=f"v{t}", name=f"v{t}") for t in range(NT)]

        # Q-phase tiles live in outer pools so their DMAs can be issued early,
        # interleaved with the KV-phase transfers on the queue.
        wqp = ctx.enter_context(tc.tile_pool(name="wq", bufs=1))
        ctxq = ctx.enter_context(tc.tile_pool(name="ctxq", bufs=1))
        wq_sb = [wqp.tile([P, D], BF, tag=f"wq{d}", name=f"wq{d}") for d in range(DN)]
        cq_sb = [ctxq.tile([P, QTOT], BF, tag=f"cq{d}", name=f"cq{d}") for d in range(DN)]

        # ---------------- K/V projection (ctx_kvT read once) ----------------
        # Two passes: all K-proj chunks first, then all V-proj.  This defers
        # the W_v deadline by ~40us, giving the startup DMA crunch (wk + early
        # ctx chunks at ~170GB/s under 8-core HBM contention) room to breathe.
        with tc.tile_pool(name="wkv", bufs=1) as wkv, \
             tc.tile_pool(name="ctxp", bufs=1) as ctxp, \
             tc.tile_pool(name="pp", bufs=8, space="PSUM") as pp:
            wk_sb = [wkv.tile([P, D], BF, tag=f"wk{d}", name=f"wk{d}") for d in range(DN)]
            wv_sb = [wkv.tile([P, D], BF, tag=f"wv{d}", name=f"wv{d}") for d in range(DN)]
            # startup-critical DMA order: W_k first halves (sync ring), first
            # ctx chunk (gpsimd ring, in parallel), then the rest chained
            HF = D // 2
            for d in range(DN):
                nc.sync.dma_start(out=wk_sb[d][:, 0:HF], in_=w_qkv[d * P:(d + 1) * P, D:D + HF])
            first_cts = []
            for d in range(DN):
                ct = ctxp.tile([P, CH], BF, tag=f"ct0_{d}", name=f"ct0_{d}")
                stage0_last = nc.gpsimd.dma_start(ct, ctx_kvT[d * P:(d + 1) * P, 0:CH])
                first_cts.append(ct)
            for d in range(DN):
                wk2 = nc.sync.dma_start(out=wk_sb[d][:, HF:D], in_=w_qkv[d * P:(d + 1) * P, D + HF:2 * D])
                add_dep_helper(wk2.ins, stage0_last.ins, sync=True, reason="dma staging")
            for d in range(DN):
                wvd = nc.sync.dma_start(out=wv_sb[d], in_=w_qkv[d * P:(d + 1) * P, 2 * D:3 * D])
                add_dep_helper(wvd.ins, wk2.ins, sync=True, reason="dma staging")
            bvd = nc.sync.dma_start(out=bv_sb, in_=bvb[:, :])
            add_dep_helper(bvd.ins, wvd.ins, sync=True, reason="dma staging")

            chunks = _chunks(N, CH)
            for ci, (coff, csz) in enumerate(chunks):
                if ci == 0:
                    cts = first_cts
                else:
                    cts = []
                    for d in range(DN):
                        ct = ctxp.tile([P, CH], BF, tag=f"ct{ci}_{d}", name=f"ct{ci}_{d}")
                        _after(nc.sync.dma_start(out=ct[:, :csz],
                                                 in_=ctx_kvT[d * P:(d + 1) * P, coff:coff + csz]), ci - 1)
                        cts.append(ct)
                    if ci == 1:        # Q-phase weights: land by ~mid-KV
                        for d in range(DN):
                            _after(nc.sync.dma_start(out=wq_sb[d], in_=w_qkv[d * P:(d + 1) * P, 0:D]), 0)
                    if ci == 2:        # Q-phase context + positions
                        for d in range(DN):
                            _after(nc.sync.dma_start(out=cq_sb[d], in_=ctx_qT[d * P:(d + 1) * P, :]), 1)
                        _after(nc.sync.dma_start(out=qpos_sb, in_=qpos[:, :]), 1)
                for e in range(DN):
                    psk = pp.tile([P, CH], F32, tag="pp8", name="psk")
                    for d in range(DN):
                        mm = nc.tensor.matmul(psk[:, :csz], lhsT=wk_sb[d][:, e * P:(e + 1) * P],
                                              rhs=cts[d][:, :csz], start=(d == 0), stop=(d == DN - 1))
                        if e == 0 and d == 0:
                            anchors.append(mm)
                        if e == DN - 1 and d == DN - 1:
                            kends.append(mm)
                    nc.scalar.activation(keT[e][:, coff:coff + csz], psk[:, :csz],
                                         AF.Identity, bias=bk_sb[:, e:e + 1], scale=1.0)
                for nt_loc in range(csz // P):
                    n_t = coff // P + nt_loc
                    for eoff, esz in _chunks(D, CH):
                        psv = pp.tile([P, CH], F32, tag="pp8", name="psv")
                        for d in range(DN):
                            nc.tensor.matmul(psv[:, :esz],
                                             lhsT=cts[d][:, nt_loc * P:(nt_loc + 1) * P],
                                             rhs=wv_sb[d][:, eoff:eoff + esz], start=(d == 0), stop=(d == DN - 1))
                        nc.vector.tensor_tensor(v_sb[n_t][:, eoff:eoff + esz], psv[:, :esz],
                                                bv_sb[:, eoff:eoff + esz], OP.add)

        # ---------------- Q projection + attention slots ----------------
        with tc.tile_pool(name="qtb", bufs=1) as qtb, \
             tc.tile_pool(name="att_e", bufs=2) as epool, \
             tc.tile_pool(name="att_m", bufs=3) as mpool, \
             tc.tile_pool(name="att_o", bufs=3) as opool, \
             tc.tile_pool(name="ps_s", bufs=2, space="PSUM") as ps_s, \
             tc.tile_pool(name="ps_pv", bufs=4, space="PSUM") as ps_pv, \
             tc.tile_pool(name="ps_den", bufs=2, space="PSUM") as ps_den:
            # Q^T for all 1024 local query rows, in [e, q] layout
            qT_sb = [qtb.tile([P, QTOT], BF, tag=f"qtb{e}", name=f"qtb{e}") for e in range(DN)]
            for qoff, qsz in _chunks(QTOT, CH):
                for e in range(DN):
                    psq = ps_s.tile([P, CH], F32, tag="s", name="psq")
                    for d in range(DN):
                        nc.tensor.matmul(psq[:, :qsz], lhsT=wq_sb[d][:, e * P:(e + 1) * P],
                                         rhs=cq_sb[d][:, qoff:qoff + qsz], start=(d == 0), stop=(d == DN - 1))
                    nc.scalar.activation(qT_sb[e][:, qoff:qoff + qsz], psq[:, :qsz],
                                         AF.Identity, bias=bq_sb[:, e:e + 1], scale=1.0)

            for s in range(NSLOT):
                KT = KS[s]
                qr0 = s * QS
                e_sb = [epool.tile([P, QS], BF, tag=f"e{k}", name=f"e{k}") for k in range(KT)]
                # scores + exp (+ mask on the last MASK_TILES k-tiles)
                for k in range(KT):
                    pss = ps_s.tile([P, QS], F32, tag="s", name="pss")
                    for d in range(DN):
                        nc.tensor.matmul(pss, lhsT=keT[d][:, k * P:(k + 1) * P],
                                         rhs=qT_sb[d][:, qr0:qr0 + QS], start=(d == 0), stop=(d == DN - 1))
                    nc.scalar.activation(e_sb[k], pss, AF.Exp, scale=SCALE)
                    if k >= KT - MASK_TILES:
                        m = mpool.tile([P, QS], BF, tag="m", name="m")
                        nc.vector.tensor_scalar(m, qpos_sb[:, qr0:qr0 + QS],
                                                kpos_sb[:, k:k + 1], None, OP.is_ge)
                        nc.vector.tensor_tensor(e_sb[k], e_sb[k], m, OP.mult)
                # PV per 128-row q-tile (V is SBUF-resident: no DMA here).
                # Denominator first, then e-chunk 0 (scaled on ScalarE and
                # DMA'd while e-chunk 1 is still in the matmul pipe), then
                # e-chunk 1 (VectorE) -- keeps the end-of-kernel chain short.
                for qt in range(QS // P):
                    psd = ps_den.tile([P, 8], F32, tag="den", name="psd")
                    for k in range(KT):
                        nc.tensor.matmul(psd, lhsT=e_sb[k][:, qt * P:(qt + 1) * P], rhs=ones_sb,
                                         start=(k == 0), stop=(k == KT - 1))
                    rec = mpool.tile([P, 1], F32, tag="rec", name="rec")
                    nc.vector.reciprocal(rec, psd[:, 0:1])
                    for ei, (eoff, esz) in enumerate(_chunks(D, CH)):
                        pso = ps_pv.tile([P, CH], F32, tag="pv", name="pso")
                        for k in range(KT):
                            nc.tensor.matmul(pso[:, :esz], lhsT=e_sb[k][:, qt * P:(qt + 1) * P],
                                             rhs=v_sb[k][:, eoff:eoff + esz],
                                             start=(k == 0), stop=(k == KT - 1))
                        ot = opool.tile([P, CH], BF, tag="o", name="ot")
                        orow = out_ext[qr0 + qt * P:qr0 + (qt + 1) * P, :]
                        if ei == 0:
                            nc.scalar.activation(ot[:, :esz], pso[:, :esz], AF.Identity, scale=rec)
                            nc.gpsimd.dma_start(out=orow[:, eoff:eoff + esz], in_=ot[:, :esz])
                        else:
                            nc.vector.tensor_scalar_mul(ot[:, :esz], pso[:, :esz], rec)
                            nc.sync.dma_start(out=orow[:, eoff:eoff + esz], in_=ot[:, :esz])
    if fix_waits:
        _fix_matmul_waits(nc)
    return nc


def make_in_maps(context, W_qkv, b_qkv, n_cores=8):
    import ml_dtypes
    bf16 = ml_dtypes.bfloat16
    context = np.ascontiguousarray(np.asarray(context, np.float32))
    W_qkv = np.ascontiguousarray(np.asarray(W_qkv, np.float32))
    b_qkv = np.ascontiguousarray(np.asarray(b_qkv, np.float32))
    B, N, D = context.shape
    NT = N // P
    DN = D // P
    kpos = (np.arange(NT)[None, :] * P + np.arange(P)[:, None]).astype(np.float32)
    kpos = np.ascontiguousarray(kpos)
    bq = np.ascontiguousarray(b_qkv[0:D].reshape(DN, P).T)
    bk = np.ascontiguousarray(b_qkv[D:2 * D].reshape(DN, P).T)
    bv = np.ascontiguousarray(np.broadcast_to(b_qkv[2 * D:3 * D], (P, D)))
    w_bf = np.ascontiguousarray(W_qkv.astype(bf16))
    in_maps = []
    for c in range(n_cores):
        b, j = divmod(c, 2)
        blocks = BLOCKS[j]
        ctx_b = context[b]
        ctx_kvT = np.ascontiguousarray(ctx_b.T.astype(bf16))
        rows = np.concatenate([np.arange(i * QS, (i + 1) * QS) for i in blocks])
        ctx_qT = np.ascontiguousarray(ctx_b[rows].T.astype(bf16))
        qpos_b = np.ascontiguousarray(
            np.broadcast_to(rows.astype(np.float32), (P, rows.size)))
        in_maps.append({
            "ctx_kvT": ctx_kvT, "ctx_qT": ctx_qT, "w_qkv": w_bf,
            "qpos": qpos_b, "kpos": kpos, "bqT": bq, "bkT": bk, "bvb": bv,
            "onesd": np.ones((P, 8), bf16),
        })
    return in_maps


def assemble(results, B, N, D):
    out = np.zeros((B, N, D), np.float32)
    for c, res in enumerate(results):
        b, j = divmod(c, 2)
        o = np.asarray(res["out"], np.float32)
        for s, i in enumerate(BLOCKS[j]):
            out[b, i * QS:(i + 1) * QS] = o[s * QS:(s + 1) * QS]
    return out


def run(inputs, trace=False, **spmd_kwargs):
    context = np.asarray(inputs["context"])
    B, N, D = context.shape
    nc = build(N, D)
    in_maps = make_in_maps(context, inputs["W_qkv"], inputs["b_qkv"], n_cores=8)
    res = run_bass_kernel_spmd(nc, in_maps, core_ids=list(range(8)), trace=trace, **spmd_kwargs)
    out = assemble(res.results, B, N, D)
    return out, res


def kernel(context, W_qkv, b_qkv):
    out, _ = run({"context": context, "W_qkv": W_qkv, "b_qkv": b_qkv})
    return out
